# revision 7
# baseline (speedup 1.0000x reference)
"""Per-pixel dynamic 5x5 filtering (kernel-predicting conv) on Trainium2.

Problem: frames (4,8,256,256,1) filtered by per-pixel 25-tap kernels
core (4,256,256,200) -> pred_i (4,8,256,256,1), pred = mean over burst N.

Sharding: pure data parallel over the burst dimension N=8 -> one burst
index per NeuronCore. Each core runs the same program over its
(4,256,256) plane set; the burst mean is taken on host during unshard.

Host prep per core shard:
  - frames zero-padded to (4,260,260) so the device needs no boundary
    handling (the 5 row shifts are baked into one DMA access pattern
    with overlapping reads).
  - per-pixel kernels laid out t-major (b, y, t, x) so every DVE access
    is contiguous.

Device kernel per core, per (batch b, 128-row block r):
  - one contiguous 3.2MB DMA of the 25-tap core tile  C: (128, 25*256)
  - one DMA building 5 row-shifted padded frame copies F: (128, 5*260)
  - one DVE tensor_tensor multiply producing all 25 tap-product planes
    P[p, (i*5+j)*256 + x] = C[p, ...] * F[p, i*260 + x + j]
    (C and P walk contiguously; F walks (i, j, x) with unit strides)
  - reduction over the 25 planes (strategy selectable: single strided
    tensor_reduce on DVE, or a contiguous add tree on DVE/GPSIMD)
  - DMA store of the (128, 256) block result
"""

import os

import numpy as np

import concourse.bass as bass
import concourse.mybir as mybir
from concourse import bacc, tile
from concourse.bass_utils import run_bass_kernel_spmd

BS, N, H, W = 4, 8, 256, 256
K = 5
KK = K * K
PAD = K // 2
HP = H + 2 * PAD  # 260
WP = W + 2 * PAD  # 260
NCORES = 8
RB = 128          # rows per block (= SBUF partitions)
NBLK = H // RB    # 2

F32 = mybir.dt.float32

# vec_reduce:  single strided tensor_reduce on DVE
# gp_tree:     contiguous add tree on GPSIMD
# vec_tree:    contiguous add tree on DVE
# split_tree:  first (largest) add level on DVE, remaining levels on GPSIMD
REDUCE_MODE = os.environ.get("KC_REDUCE", "split_tree")

_nc = None


def _emit_tree_reduce(nc, eng, P):
    """Sum the 25 contiguous 256-elem planes of P into P[:, 0:W]."""
    add = mybir.AluOpType.add
    nplanes = KK
    while nplanes > 1:
        half = nplanes // 2          # planes in each added half
        lo = P[:, 0:half * W]
        hi = P[:, half * W:2 * half * W]
        eng.tensor_tensor(lo, lo, hi, add)
        if nplanes % 2:              # fold the odd leftover plane into plane 0
            last = P[:, (nplanes - 1) * W:nplanes * W]
            eng.tensor_tensor(P[:, 0:W], P[:, 0:W], last, add)
        nplanes = half


def _build_nc():
    nc = bacc.Bacc(
        "TRN2",
        target_bir_lowering=False,
        debug=False,
        num_devices=NCORES,
    )
    fr_h = nc.dram_tensor("fr_pad", [BS, HP, WP], F32, kind="ExternalInput")
    co_h = nc.dram_tensor("co_s", [BS, H, KK * W], F32, kind="ExternalInput")
    pr_h = nc.dram_tensor("pred_s", [BS, H, W], F32, kind="ExternalOutput")
    co = co_h.ap()
    pr = pr_h.ap()

    mult = mybir.AluOpType.mult
    add = mybir.AluOpType.add

    with tile.TileContext(nc) as tc:
        with (
            tc.tile_pool(name="cpool", bufs=4) as cpool,
            tc.tile_pool(name="fpool", bufs=4) as fpool,
            tc.tile_pool(name="ppool", bufs=2) as ppool,
            tc.tile_pool(name="apool", bufs=3) as apool,
        ):
            for b in range(BS):
                for r in range(NBLK):
                    C = cpool.tile([RB, KK * W], F32, name=f"C_{b}_{r}", tag="C")
                    nc.sync.dma_start(out=C, in_=co[b, r * RB:(r + 1) * RB, :])

                    # F[p, i*WP + cc] = fr[b, r*RB + p + i, cc]
                    F = fpool.tile([RB, K * WP], F32, name=f"F_{b}_{r}", tag="F")
                    src = bass.AP(
                        fr_h,
                        (b * HP + r * RB) * WP,
                        [[WP, RB], [WP, K], [1, WP]],
                    )
                    nc.gpsimd.dma_start(out=F, in_=src)

                    # P[p, (i*5+j)*W + x] = C[p, same] * F[p, i*WP + x + j]
                    P = ppool.tile([RB, KK * W], F32, name=f"P_{b}_{r}", tag="P")
                    c_v = C.rearrange("p (i j x) -> p i j x", i=K, j=K)
                    p_v = P.rearrange("p (i j x) -> p i j x", i=K, j=K)
                    f_v = bass.AP(
                        F.tensor,
                        F.offset,
                        [list(F.ap[0]), [WP, K], [1, K], [1, W]],
                    )
                    nc.vector.tensor_tensor(p_v, c_v, f_v, mult)

                    if REDUCE_MODE == "vec_reduce":
                        A = apool.tile([RB, W], F32, name=f"A_{b}_{r}", tag="A")
                        nc.vector.tensor_reduce(
                            A[:],
                            P.rearrange("p (t x) -> p x t", t=KK),
                            axis=mybir.AxisListType.X,
                            op=add,
                        )
                        out_src = A[:]
                    elif REDUCE_MODE == "gp_tree":
                        _emit_tree_reduce(nc, nc.gpsimd, P)
                        out_src = P[:, 0:W]
                    elif REDUCE_MODE == "vec_tree":
                        _emit_tree_reduce(nc, nc.vector, P)
                        out_src = P[:, 0:W]
                    elif REDUCE_MODE == "split_tree":
                        # DVE folds planes 12..23 into 0..11 (one big
                        # contiguous add); GPSIMD finishes 12 -> 1 plus the
                        # leftover plane 24.
                        nc.vector.tensor_tensor(
                            P[:, 0:12 * W], P[:, 0:12 * W],
                            P[:, 12 * W:24 * W], add,
                        )
                        g = nc.gpsimd
                        g.tensor_tensor(
                            P[:, 0:6 * W], P[:, 0:6 * W],
                            P[:, 6 * W:12 * W], add,
                        )
                        g.tensor_tensor(
                            P[:, 0:3 * W], P[:, 0:3 * W],
                            P[:, 3 * W:6 * W], add,
                        )
                        g.tensor_tensor(
                            P[:, 0:W], P[:, 0:W], P[:, W:2 * W], add,
                        )
                        g.tensor_tensor(
                            P[:, 0:W], P[:, 0:W], P[:, 2 * W:3 * W], add,
                        )
                        g.tensor_tensor(
                            P[:, 0:W], P[:, 0:W], P[:, 24 * W:25 * W], add,
                        )
                        out_src = P[:, 0:W]
                    else:
                        raise ValueError(REDUCE_MODE)

                    # Keep the big C loads alone on the nc.sync HWDGE FIFO;
                    # frame loads and stores ride SWDGE (gpsimd) so nothing
                    # ever head-of-line blocks a C prefetch.
                    nc.gpsimd.dma_start(
                        out=pr[b, r * RB:(r + 1) * RB, :], in_=out_src
                    )
    nc.compile()
    return nc


def _get_nc():
    global _nc
    if _nc is None:
        _nc = _build_nc()
    return _nc


def _prepare_in_maps(frames, core):
    frames = np.ascontiguousarray(np.asarray(frames, dtype=np.float32))
    core = np.ascontiguousarray(np.asarray(core, dtype=np.float32))
    # Same raw reinterpretation the reference does:
    # (bs, h, w, N*c*K*K) -> (bs, N, h, w, c*K*K)
    core_k = core.reshape(BS, N, H, W, KK)
    in_maps = []
    for j in range(NCORES):
        fp = np.zeros((BS, HP, WP), np.float32)
        fp[:, PAD:PAD + H, PAD:PAD + W] = frames[:, j, :, :, 0]
        # t-major device layout: (b, y, t, x)
        cs = np.ascontiguousarray(
            core_k[:, j].transpose(0, 1, 3, 2)
        ).reshape(BS, H, KK * W)
        in_maps.append({"fr_pad": fp, "co_s": cs})
    return in_maps


def _gather(results):
    pred_i = np.stack([results[j]["pred_s"] for j in range(NCORES)], axis=1)
    pred_img_i = np.ascontiguousarray(pred_i[..., None].astype(np.float32))
    pred_img = pred_img_i.mean(axis=1, dtype=np.float32)
    return pred_img, pred_img_i


def kernel(frames, core):
    nc = _get_nc()
    in_maps = _prepare_in_maps(frames, core)
    res = run_bass_kernel_spmd(nc, in_maps, list(range(NCORES))).results
    return _gather(res)


# revision 10
# speedup vs baseline: 1.3371x; 1.3371x over previous
"""Per-pixel dynamic 5x5 filtering (kernel-predicting conv) on Trainium2.

Problem: frames (4,8,256,256,1) filtered by per-pixel 25-tap kernels
core (4,256,256,200) -> pred_i (4,8,256,256,1), pred = mean over burst N.

Sharding: pure data parallel over the burst dimension N=8 -> one burst
index per NeuronCore. Each core runs the same program over its
(4,256,256) plane set; the burst mean is taken on host during unshard.

Host prep per core shard:
  - frames zero-padded to (4,260,260) so the device needs no boundary
    handling (the 5 row shifts are baked into one DMA access pattern
    with overlapping reads).
  - per-pixel kernels laid out t-major (b, y, t, x) so every DVE access
    is contiguous.

Device kernel per core, per (batch b, 128-row block r):
  - one contiguous 3.2MB DMA of the 25-tap core tile  C: (128, 25*256)
  - one DMA building 5 row-shifted padded frame copies F: (128, 5*260)
  - one DVE tensor_tensor multiply producing all 25 tap-product planes
    P[p, (i*5+j)*256 + x] = C[p, ...] * F[p, i*260 + x + j]
    (C and P walk contiguously; F walks (i, j, x) with unit strides)
  - reduction over the 25 planes (strategy selectable: single strided
    tensor_reduce on DVE, or a contiguous add tree on DVE/GPSIMD)
  - DMA store of the (128, 256) block result
"""

import os

import numpy as np

import concourse.bass as bass
import concourse.mybir as mybir
from concourse import bacc, tile
from concourse.bass_utils import run_bass_kernel_spmd

BS, N, H, W = 4, 8, 256, 256
K = 5
KK = K * K
PAD = K // 2
HP = H + 2 * PAD  # 260
WP = W + 2 * PAD  # 260
NCORES = 8
RB = 128          # rows per block (= SBUF partitions)
NBLK = H // RB    # 2

F32 = mybir.dt.float32
BF16 = mybir.dt.bfloat16

# vec_reduce:  single strided tensor_reduce on DVE
# gp_tree:     contiguous add tree on GPSIMD
# vec_tree:    contiguous add tree on DVE
# split_tree:  first (largest) add level on DVE, remaining levels on GPSIMD
REDUCE_MODE = os.environ.get("KC_REDUCE", "vec_tree")
# f32 (exact) or bf16 (2x DVE mode + half the core-tensor DMA traffic)
DTYPE = os.environ.get("KC_DTYPE", "f32")

_nc = None


def _emit_tree_reduce(nc, eng, P, final_out=None):
    """Sum the 25 contiguous 256-elem planes of P into P[:, 0:W] (or write
    the final 256-elem accumulation into final_out, e.g. an fp32 tile)."""
    add = mybir.AluOpType.add
    ops = []
    nplanes = KK
    while nplanes > 1:
        half = nplanes // 2          # planes in each added half
        ops.append((P[:, 0:half * W], P[:, 0:half * W],
                    P[:, half * W:2 * half * W]))
        if nplanes % 2:              # fold the odd leftover plane into plane 0
            last = P[:, (nplanes - 1) * W:nplanes * W]
            ops.append((P[:, 0:W], P[:, 0:W], last))
        nplanes = half
    if final_out is not None:
        out, in0, in1 = ops[-1]
        ops[-1] = (final_out, in0, in1)
    for out, in0, in1 in ops:
        eng.tensor_tensor(out, in0, in1, add)


def _build_nc():
    nc = bacc.Bacc(
        "TRN2",
        target_bir_lowering=False,
        debug=False,
        num_devices=NCORES,
    )
    DT = BF16 if DTYPE == "bf16" else F32
    fr_h = nc.dram_tensor("fr_pad", [BS, HP, WP], DT, kind="ExternalInput")
    co_h = nc.dram_tensor("co_s", [BS, H, KK * W], DT, kind="ExternalInput")
    pr_h = nc.dram_tensor("pred_s", [BS, H, W], F32, kind="ExternalOutput")
    co = co_h.ap()
    pr = pr_h.ap()

    mult = mybir.AluOpType.mult
    add = mybir.AluOpType.add

    with tile.TileContext(nc) as tc:
        with (
            tc.tile_pool(name="cpool", bufs=4) as cpool,
            tc.tile_pool(name="fpool", bufs=4) as fpool,
            tc.tile_pool(name="ppool", bufs=2) as ppool,
            tc.tile_pool(name="apool", bufs=3) as apool,
        ):
            for b in range(BS):
                for r in range(NBLK):
                    C = cpool.tile([RB, KK * W], DT, name=f"C_{b}_{r}", tag="C")
                    nc.sync.dma_start(out=C, in_=co[b, r * RB:(r + 1) * RB, :])

                    # F[p, i*WP + cc] = fr[b, r*RB + p + i, cc]
                    F = fpool.tile([RB, K * WP], DT, name=f"F_{b}_{r}", tag="F")
                    src = bass.AP(
                        fr_h,
                        (b * HP + r * RB) * WP,
                        [[WP, RB], [WP, K], [1, WP]],
                    )
                    nc.sync.dma_start(out=F, in_=src)

                    # P[p, (i*5+j)*W + x] = C[p, same] * F[p, i*WP + x + j]
                    P = ppool.tile([RB, KK * W], DT, name=f"P_{b}_{r}", tag="P")
                    c_v = C.rearrange("p (i j x) -> p i j x", i=K, j=K)
                    p_v = P.rearrange("p (i j x) -> p i j x", i=K, j=K)
                    f_v = bass.AP(
                        F.tensor,
                        F.offset,
                        [list(F.ap[0]), [WP, K], [1, K], [1, W]],
                    )
                    nc.vector.tensor_tensor(p_v, c_v, f_v, mult)

                    if REDUCE_MODE == "vec_reduce":
                        A = apool.tile([RB, W], F32, name=f"A_{b}_{r}", tag="A")
                        nc.vector.tensor_reduce(
                            A[:],
                            P.rearrange("p (t x) -> p x t", t=KK),
                            axis=mybir.AxisListType.X,
                            op=add,
                        )
                        out_src = A[:]
                    elif REDUCE_MODE == "gp_tree":
                        _emit_tree_reduce(nc, nc.gpsimd, P)
                        out_src = P[:, 0:W]
                    elif REDUCE_MODE == "vec_tree":
                        # Final add lands in the small fp32 A tile: the store
                        # then reads A, so P's big slot frees for the next
                        # block as soon as the tree finishes (and in bf16 mode
                        # this up-converts the result for free).
                        A = apool.tile([RB, W], F32, name=f"A_{b}_{r}", tag="A")
                        _emit_tree_reduce(nc, nc.vector, P, final_out=A[:])
                        out_src = A[:]
                    elif REDUCE_MODE == "split_tree":
                        # DVE folds planes 12..23 into 0..11 (one big
                        # contiguous add); GPSIMD finishes 12 -> 1 plus the
                        # leftover plane 24.
                        nc.vector.tensor_tensor(
                            P[:, 0:12 * W], P[:, 0:12 * W],
                            P[:, 12 * W:24 * W], add,
                        )
                        g = nc.gpsimd
                        g.tensor_tensor(
                            P[:, 0:6 * W], P[:, 0:6 * W],
                            P[:, 6 * W:12 * W], add,
                        )
                        g.tensor_tensor(
                            P[:, 0:3 * W], P[:, 0:3 * W],
                            P[:, 3 * W:6 * W], add,
                        )
                        g.tensor_tensor(
                            P[:, 0:W], P[:, 0:W], P[:, W:2 * W], add,
                        )
                        g.tensor_tensor(
                            P[:, 0:W], P[:, 0:W], P[:, 2 * W:3 * W], add,
                        )
                        g.tensor_tensor(
                            P[:, 0:W], P[:, 0:W], P[:, 24 * W:25 * W], add,
                        )
                        out_src = P[:, 0:W]
                    else:
                        raise ValueError(REDUCE_MODE)

                    nc.sync.dma_start(
                        out=pr[b, r * RB:(r + 1) * RB, :], in_=out_src
                    )
    nc.compile()
    return nc


def _get_nc():
    global _nc
    if _nc is None:
        _nc = _build_nc()
    return _nc


def _prepare_in_maps(frames, core):
    frames = np.ascontiguousarray(np.asarray(frames, dtype=np.float32))
    core = np.ascontiguousarray(np.asarray(core, dtype=np.float32))
    if DTYPE == "bf16":
        import ml_dtypes
        np_dt = np.dtype(ml_dtypes.bfloat16)
    else:
        np_dt = np.float32
    # Same raw reinterpretation the reference does:
    # (bs, h, w, N*c*K*K) -> (bs, N, h, w, c*K*K)
    core_k = core.reshape(BS, N, H, W, KK)
    in_maps = []
    for j in range(NCORES):
        fp = np.zeros((BS, HP, WP), np_dt)
        fp[:, PAD:PAD + H, PAD:PAD + W] = frames[:, j, :, :, 0].astype(np_dt)
        # t-major device layout: (b, y, t, x)
        cs = np.ascontiguousarray(
            core_k[:, j].transpose(0, 1, 3, 2).astype(np_dt)
        ).reshape(BS, H, KK * W)
        in_maps.append({"fr_pad": fp, "co_s": cs})
    return in_maps


def _gather(results):
    pred_i = np.stack([results[j]["pred_s"] for j in range(NCORES)], axis=1)
    pred_img_i = np.ascontiguousarray(pred_i[..., None].astype(np.float32))
    pred_img = pred_img_i.mean(axis=1, dtype=np.float32)
    return pred_img, pred_img_i


def kernel(frames, core):
    nc = _get_nc()
    in_maps = _prepare_in_maps(frames, core)
    res = run_bass_kernel_spmd(nc, in_maps, list(range(NCORES))).results
    return _gather(res)


# revision 12
# speedup vs baseline: 1.3436x; 1.0048x over previous
"""Per-pixel dynamic 5x5 filtering (kernel-predicting conv) on Trainium2.

Problem: frames (4,8,256,256,1) filtered by per-pixel 25-tap kernels
core (4,256,256,200) -> pred_i (4,8,256,256,1), pred = mean over burst N.

Sharding: pure data parallel over the burst dimension N=8 -> one burst
index per NeuronCore. Each core runs the same program over its
(4,256,256) plane set; the burst mean is taken on host during unshard.

Host prep per core shard:
  - frames zero-padded to (4,260,260) so the device needs no boundary
    handling (the 5 row shifts are baked into one DMA access pattern
    with overlapping reads).
  - per-pixel kernels laid out t-major (b, y, t, x) so every DVE access
    is contiguous.

Device kernel per core, per (batch b, 128-row block r):
  - one contiguous 3.2MB DMA of the 25-tap core tile  C: (128, 25*256)
  - one DMA building 5 row-shifted padded frame copies F: (128, 5*260)
  - one DVE tensor_tensor multiply producing all 25 tap-product planes
    P[p, (i*5+j)*256 + x] = C[p, ...] * F[p, i*260 + x + j]
    (C and P walk contiguously; F walks (i, j, x) with unit strides)
  - reduction over the 25 planes (strategy selectable: single strided
    tensor_reduce on DVE, or a contiguous add tree on DVE/GPSIMD)
  - DMA store of the (128, 256) block result
"""

import os

import numpy as np

import concourse.bass as bass
import concourse.mybir as mybir
from concourse import bacc, tile
from concourse.bass_utils import run_bass_kernel_spmd

BS, N, H, W = 4, 8, 256, 256
K = 5
KK = K * K
PAD = K // 2
HP = H + 2 * PAD  # 260
WP = W + 2 * PAD  # 260
NCORES = 8
RB = 128          # rows per block (= SBUF partitions)
NBLK = H // RB    # 2

F32 = mybir.dt.float32
BF16 = mybir.dt.bfloat16

# vec_reduce:  single strided tensor_reduce on DVE
# gp_tree:     contiguous add tree on GPSIMD
# vec_tree:    contiguous add tree on DVE
# split_tree:  first (largest) add level on DVE, remaining levels on GPSIMD
REDUCE_MODE = os.environ.get("KC_REDUCE", "vec_tree")
# f32 (exact) or bf16 (2x DVE mode + half the core-tensor DMA traffic)
DTYPE = os.environ.get("KC_DTYPE", "f32")

_nc = None


def _emit_tree_reduce(nc, eng, P, final_out=None):
    """Sum the 25 contiguous 256-elem planes of P into P[:, 0:W] (or write
    the final 256-elem accumulation into final_out, e.g. an fp32 tile)."""
    add = mybir.AluOpType.add
    ops = []
    nplanes = KK
    while nplanes > 1:
        half = nplanes // 2          # planes in each added half
        ops.append((P[:, 0:half * W], P[:, 0:half * W],
                    P[:, half * W:2 * half * W]))
        if nplanes % 2:              # fold the odd leftover plane into plane 0
            last = P[:, (nplanes - 1) * W:nplanes * W]
            ops.append((P[:, 0:W], P[:, 0:W], last))
        nplanes = half
    if final_out is not None:
        out, in0, in1 = ops[-1]
        ops[-1] = (final_out, in0, in1)
    for out, in0, in1 in ops:
        eng.tensor_tensor(out, in0, in1, add)


def _build_nc():
    nc = bacc.Bacc(
        "TRN2",
        target_bir_lowering=False,
        debug=False,
        num_devices=NCORES,
    )
    DT = BF16 if DTYPE == "bf16" else F32
    fr_h = nc.dram_tensor("fr_pad", [BS, HP, WP], DT, kind="ExternalInput")
    co_h = nc.dram_tensor("co_s", [BS, H, KK * W], DT, kind="ExternalInput")
    pr_h = nc.dram_tensor("pred_s", [BS, H, W], F32, kind="ExternalOutput")
    co = co_h.ap()
    pr = pr_h.ap()

    mult = mybir.AluOpType.mult
    add = mybir.AluOpType.add

    with tile.TileContext(nc) as tc:
        with (
            tc.tile_pool(name="cpool", bufs=4) as cpool,
            tc.tile_pool(name="fpool", bufs=4) as fpool,
            tc.tile_pool(name="ppool", bufs=2) as ppool,
            tc.tile_pool(name="apool", bufs=3) as apool,
        ):
            first = True
            for b in range(BS):
                for r in range(NBLK):
                    # F[p, i*WP + cc] = fr[b, r*RB + p + i, cc]
                    F = fpool.tile([RB, K * WP], DT, name=f"F_{b}_{r}", tag="F")
                    src = bass.AP(
                        fr_h,
                        (b * HP + r * RB) * WP,
                        [[WP, RB], [WP, K], [1, WP]],
                    )
                    nc.sync.dma_start(out=F, in_=src)

                    # P[p, (i*5+j)*W + x] = C[p, same] * F[p, i*WP + x + j]
                    C = cpool.tile([RB, KK * W], DT, name=f"C_{b}_{r}", tag="C")
                    P = ppool.tile([RB, KK * W], DT, name=f"P_{b}_{r}", tag="P")
                    c_v = C.rearrange("p (i j x) -> p i j x", i=K, j=K)
                    p_v = P.rearrange("p (i j x) -> p i j x", i=K, j=K)
                    if first:
                        # Split the pipeline-filling first block into per-i
                        # chunks so DVE starts after a 640KB load instead of
                        # waiting out the full 3.2MB tile.
                        first = False
                        for i in range(K):
                            seg = slice(i * K * W, (i + 1) * K * W)
                            nc.sync.dma_start(
                                out=C[:, seg], in_=co[b, r * RB:(r + 1) * RB, seg]
                            )
                            f_vi = bass.AP(
                                F.tensor,
                                F.offset + i * WP,
                                [list(F.ap[0]), [1, K], [1, W]],
                            )
                            nc.vector.tensor_tensor(
                                p_v[:, i], c_v[:, i], f_vi, mult
                            )
                    else:
                        nc.sync.dma_start(
                            out=C, in_=co[b, r * RB:(r + 1) * RB, :]
                        )
                        f_v = bass.AP(
                            F.tensor,
                            F.offset,
                            [list(F.ap[0]), [WP, K], [1, K], [1, W]],
                        )
                        nc.vector.tensor_tensor(p_v, c_v, f_v, mult)

                    if REDUCE_MODE == "vec_reduce":
                        A = apool.tile([RB, W], F32, name=f"A_{b}_{r}", tag="A")
                        nc.vector.tensor_reduce(
                            A[:],
                            P.rearrange("p (t x) -> p x t", t=KK),
                            axis=mybir.AxisListType.X,
                            op=add,
                        )
                        out_src = A[:]
                    elif REDUCE_MODE == "gp_tree":
                        _emit_tree_reduce(nc, nc.gpsimd, P)
                        out_src = P[:, 0:W]
                    elif REDUCE_MODE == "vec_tree":
                        # Contiguous 5-op add tree on DVE. The 4th op sums
                        # plane pairs {0,1} += {2,24} via a strided-pair AP;
                        # the final add lands in the small fp32 A tile so the
                        # store never holds P's big slot (and in bf16 mode
                        # this up-converts the result for free).
                        A = apool.tile([RB, W], F32, name=f"A_{b}_{r}", tag="A")
                        v = nc.vector
                        v.tensor_tensor(P[:, 0:12 * W], P[:, 0:12 * W],
                                        P[:, 12 * W:24 * W], add)
                        v.tensor_tensor(P[:, 0:6 * W], P[:, 0:6 * W],
                                        P[:, 6 * W:12 * W], add)
                        v.tensor_tensor(P[:, 0:3 * W], P[:, 0:3 * W],
                                        P[:, 3 * W:6 * W], add)
                        pair = bass.AP(
                            P.tensor, P.offset + 2 * W,
                            [list(P.ap[0]), [22 * W, 2], [1, W]],
                        )
                        v.tensor_tensor(P[:, 0:2 * W], P[:, 0:2 * W], pair, add)
                        v.tensor_tensor(A[:], P[:, 0:W], P[:, W:2 * W], add)
                        out_src = A[:]
                    elif REDUCE_MODE == "split_tree":
                        # DVE folds planes 12..23 into 0..11 (one big
                        # contiguous add); GPSIMD finishes 12 -> 1 plus the
                        # leftover plane 24.
                        nc.vector.tensor_tensor(
                            P[:, 0:12 * W], P[:, 0:12 * W],
                            P[:, 12 * W:24 * W], add,
                        )
                        g = nc.gpsimd
                        g.tensor_tensor(
                            P[:, 0:6 * W], P[:, 0:6 * W],
                            P[:, 6 * W:12 * W], add,
                        )
                        g.tensor_tensor(
                            P[:, 0:3 * W], P[:, 0:3 * W],
                            P[:, 3 * W:6 * W], add,
                        )
                        g.tensor_tensor(
                            P[:, 0:W], P[:, 0:W], P[:, W:2 * W], add,
                        )
                        g.tensor_tensor(
                            P[:, 0:W], P[:, 0:W], P[:, 2 * W:3 * W], add,
                        )
                        g.tensor_tensor(
                            P[:, 0:W], P[:, 0:W], P[:, 24 * W:25 * W], add,
                        )
                        out_src = P[:, 0:W]
                    else:
                        raise ValueError(REDUCE_MODE)

                    nc.sync.dma_start(
                        out=pr[b, r * RB:(r + 1) * RB, :], in_=out_src
                    )
    nc.compile()
    return nc


def _get_nc():
    global _nc
    if _nc is None:
        _nc = _build_nc()
    return _nc


def _prepare_in_maps(frames, core):
    frames = np.ascontiguousarray(np.asarray(frames, dtype=np.float32))
    core = np.ascontiguousarray(np.asarray(core, dtype=np.float32))
    if DTYPE == "bf16":
        import ml_dtypes
        np_dt = np.dtype(ml_dtypes.bfloat16)
    else:
        np_dt = np.float32
    # Same raw reinterpretation the reference does:
    # (bs, h, w, N*c*K*K) -> (bs, N, h, w, c*K*K)
    core_k = core.reshape(BS, N, H, W, KK)
    in_maps = []
    for j in range(NCORES):
        fp = np.zeros((BS, HP, WP), np_dt)
        fp[:, PAD:PAD + H, PAD:PAD + W] = frames[:, j, :, :, 0].astype(np_dt)
        # t-major device layout: (b, y, t, x)
        cs = np.ascontiguousarray(
            core_k[:, j].transpose(0, 1, 3, 2).astype(np_dt)
        ).reshape(BS, H, KK * W)
        in_maps.append({"fr_pad": fp, "co_s": cs})
    return in_maps


def _gather(results):
    pred_i = np.stack([results[j]["pred_s"] for j in range(NCORES)], axis=1)
    pred_img_i = np.ascontiguousarray(pred_i[..., None].astype(np.float32))
    pred_img = pred_img_i.mean(axis=1, dtype=np.float32)
    return pred_img, pred_img_i


def kernel(frames, core):
    nc = _get_nc()
    in_maps = _prepare_in_maps(frames, core)
    res = run_bass_kernel_spmd(nc, in_maps, list(range(NCORES))).results
    return _gather(res)
